# revision 1
# baseline (speedup 1.0000x reference)
"""Trainium2 Bass kernel for nn_AttentionRNN (embedding + masked GRU + MLP head + softmax).

Strategy (pure data parallelism over 8 NeuronCores, 2048 examples/core):

Layout: everything transposed — state h kept as hT [H=128 partitions, examples
on free dim], so the GRU recurrence is closed under the layout (no per-step
transposes). Per time step t, per 512-example group (psum tile [128, 1536] f32):

  psum[:, 0:512]    = U_z.T @ hT + M_z.T @ xghT + 1s.T @ notm_t   (z preact)
  psum[:, 512:1024] = U_r.T @ hT + M_r.T @ xghT                   (r preact)
  psum[:, 1024:1536]= U_h.T @ hT                                  (rec_h)
  z|r  = sigmoid(psum[:, 0:1024])                 (one ACT call, reads PSUM)
  t1   = (rec_h + b1_h) * r                       (fused DVE scalar_tensor_tensor)
  t2   = t1 + xgh
  hh   = tanh(t2)
  h'   = z*(h - hh) + hh                          (3 DVE tensor_tensor ops)

The only gather is xghT: dma_gather (transpose mode) from a host-precomputed
fp16 table gtab[V, 128] = emb @ W[:, 256:384] + b0_h  (256B rows).  The z/r
input projections are reconstructed algebraically instead of gathered:
  x = (xgh - b0_h) @ pinv(W_h)  (exact: xgh lies in W_h's 32-dim row space)
  xg_zr = x @ W_zr = xgh @ M + c,  M = pinv(W_h) @ W_zr  (host-precomputed)
so no second gather is needed.  dma_gather is HW-limited to ~896 idxs/call
(1024+ kills the Pool engine), so each step gathers in (896, 896, 256) splits.

Mask (token==0 freezes state): notmT[t, i] = 100 if token==0 else 0, shipped
from host; a K=1 matmul adds it to the z preactivation => z = sigmoid(.+100) = 1
exactly => h' = h.  Biases: b0_h folded into gtab; b1_h via the STT scalar;
b0/b1_zr (+ the -b0_h@M correction) via K=1 matmuls only when nonzero.

Head: dT = swish(W1.T @ hT + b1); logits per 128-example tile with examples on
partitions (lhsT = dT slice); softmax along free dim (C=3).
"""

import numpy as np
from contextlib import ExitStack

import concourse.mybir as mybir
import concourse.tile as tile
from concourse import bacc
from concourse.bass_utils import run_bass_kernel_spmd

B, T, E, H, V, D, C = 16384, 128, 32, 128, 30001, 128, 3
NCORES = 8
BC = B // NCORES
BIGM = 100.0
NIDX = 896             # max idxs per dma_gather call (HW-probed ucode limit:
                       # 896 works, 1024+ crashes the Pool engine)
USE_SPLIT = True       # split zr/g psum tiles vs one 3-bank tile
G_BUFS = 3
H_BUFS = 3
Z_BUFS = 3
TMP_BUFS = 3
NM_CH = 4
PW = 512
SIG_SPLIT = False
TANH_MERGE = False
HEAD_SHARE_PS = False
R_FIRST = False
SKIP_GATHER = False
GSPLIT_OVERRIDE = None
PSZR_BUFS = 2          # psum pool depths (8 banks total: 2*PSZR + PSG + PH)
PSG_BUFS = 2
PH_BUFS = 2

F16 = mybir.dt.float16
F32 = mybir.dt.float32
I16 = mybir.dt.int16
AF = mybir.ActivationFunctionType
OP = mybir.AluOpType
AX = mybir.AxisListType


def build_nc(bc=BC, nt=T, with_czr=False):
    """Build + compile the per-core Bass program. bc = examples per core."""
    assert bc % 512 == 0
    ng = bc // 512            # 512-example groups per step
    pw = min(PW, bc)          # width of the wide DVE ops
    npairs = bc // pw
    gperp = pw // 512
    gsplit = []
    off = 0
    while off < bc:
        n = min(NIDX, bc - off)
        gsplit.append((off, n))
        off += n
    if GSPLIT_OVERRIDE:
        gsplit = GSPLIT_OVERRIDE

    nc = bacc.Bacc("TRN2", target_bir_lowering=False, debug=False)
    gtab = nc.dram_tensor("gtab", [V, 128], F16, kind="ExternalInput").ap()
    idxw = nc.dram_tensor("idxw", [128, nt * bc // 16], I16, kind="ExternalInput").ap()
    uzrh = nc.dram_tensor("uzrh", [128, 384], F16, kind="ExternalInput").ap()
    mzr = nc.dram_tensor("mzr", [128, 256], F16, kind="ExternalInput").ap()
    notm = nc.dram_tensor("notm", [1, nt * bc], F16, kind="ExternalInput").ap()
    b1h = nc.dram_tensor("b1h", [128, 1], F32, kind="ExternalInput").ap()
    w1 = nc.dram_tensor("w1", [128, 128], F16, kind="ExternalInput").ap()
    b1c = nc.dram_tensor("b1c", [128, 1], F32, kind="ExternalInput").ap()
    wout = nc.dram_tensor("wout", [128, C], F16, kind="ExternalInput").ap()
    boutw = nc.dram_tensor("boutw", [1, C], F16, kind="ExternalInput").ap()
    if with_czr:
        czr = nc.dram_tensor("czr", [1, 256], F16, kind="ExternalInput").ap()
    outp = nc.dram_tensor("outp", [128, (bc // 128) * C], F32, kind="ExternalOutput").ap()

    with tile.TileContext(nc) as tc, ExitStack() as ctx:
        wp = ctx.enter_context(tc.tile_pool(name="w", bufs=1))
        ip = ctx.enter_context(tc.tile_pool(name="idx", bufs=1))
        gp = ctx.enter_context(tc.tile_pool(name="g", bufs=G_BUFS))
        hp = ctx.enter_context(tc.tile_pool(name="h", bufs=H_BUFS))
        zp = ctx.enter_context(tc.tile_pool(name="zr", bufs=Z_BUFS))
        tp = ctx.enter_context(tc.tile_pool(name="tmp", bufs=TMP_BUFS))
        pzr = ctx.enter_context(tc.tile_pool(name="pszr", bufs=PSZR_BUFS, space="PSUM"))
        pg = ctx.enter_context(tc.tile_pool(name="psg", bufs=PSG_BUFS, space="PSUM"))
        hd = ctx.enter_context(tc.tile_pool(name="hd", bufs=2))
        ph = ctx.enter_context(tc.tile_pool(name="ph", bufs=PH_BUFS, space="PSUM"))

        u_sb = wp.tile([128, 384], F16, tag="u")
        nc.sync.dma_start(u_sb[:], uzrh)
        m_sb = wp.tile([128, 256], F16, tag="mzr")
        nc.sync.dma_start(m_sb[:], mzr)
        b1h_sb = wp.tile([128, 1], F32, tag="b1h")
        nc.sync.dma_start(b1h_sb[:], b1h)
        w1_sb = wp.tile([128, 128], F16, tag="w1")
        nc.sync.dma_start(w1_sb[:], w1)
        b1c_sb = wp.tile([128, 1], F32, tag="b1c")
        nc.sync.dma_start(b1c_sb[:], b1c)
        wout_sb = wp.tile([128, C], F16, tag="wo")
        nc.sync.dma_start(wout_sb[:], wout)
        bout_sb = wp.tile([1, C], F16, tag="bo")
        nc.sync.dma_start(bout_sb[:], boutw)
        ones_sb = wp.tile([1, 128], F16, tag="ones")
        nc.vector.memset(ones_sb[:], 1.0)
        # Pin the ACT table set that contains BOTH Sigmoid and Tanh so the
        # auto-placement pass doesn't ping-pong table loads every step
        # (~1.3us per load on the ACT critical path).
        from concourse.hw_specs import get_activation_tables
        _tabs = get_activation_tables(nc.m.arch)
        _setid = next(i for i, (nm2, fs) in enumerate(_tabs.items())
                      if AF.Sigmoid in fs and AF.Tanh in fs)
        nc.scalar.add_instruction(mybir.InstLoadActFuncSet(
            name=nc.get_next_instruction_name(), ins=[], outs=[],
            act_func_set_id=_setid))
        if with_czr:
            czr_sb = wp.tile([1, 256], F16, tag="czr")
            nc.sync.dma_start(czr_sb[:], czr)
            onesbc_sb = wp.tile([1, bc], F16, tag="onesbc")
            nc.vector.memset(onesbc_sb[:], 1.0)
        idx_sb = ip.tile([128, nt * bc // 16], I16, tag="idx")
        nc.sync.dma_start(idx_sb[:], idxw)

        nmp = ctx.enter_context(tc.tile_pool(name="nm", bufs=2))
        NMCH = NM_CH      # timesteps of notm per staged chunk
        h = hp.tile([128, bc], F16, tag="h")
        nc.vector.memset(h[:], 0.0)

        nm_sb = None
        for t in range(nt):
            if t % NMCH == 0:
                nm_sb = nmp.tile([1, NMCH * bc], F16, tag="nm")
                nc.sync.dma_start(nm_sb[:], notm[:, t * bc:(t + NMCH) * bc])
            g = gp.tile([128, 1, bc], F16, tag="g")
            if SKIP_GATHER:       # timing-probe flag: cheap gpsimd fill instead
                nc.gpsimd.memset(g[:], 0.01)
            else:
                for off, n in gsplit:
                    nc.gpsimd.dma_gather(
                        g[:, :, off:off + n], gtab,
                        idx_sb[:, (t * bc + off) // 16:(t * bc + off + n) // 16],
                        n, n, 128, transpose=True,
                    )
            xgh = g[:, 0, :]
            zr = zp.tile([128, 2 * bc], F16, tag="zr")
            t1 = tp.tile([128, bc], F16, tag="t1")
            hnew = hp.tile([128, bc], F16, tag="h")
            for gi in range(ng):
                if USE_SPLIT:
                    ps_t = pzr.tile([128, 1024], F32, tag="ps")
                    pG_t = pg.tile([128, 512], F32, tag="pg")
                    ps = ps_t[:]
                    pG = pG_t[:]
                else:
                    both = pzr.tile([128, 1536], F32, tag="ps")
                    ps = both[:, 0:1024]
                    pG = both[:, 1024:1536]
                exs = slice(gi * 512, (gi + 1) * 512)
                def z_mms():
                    nc.tensor.matmul(ps[:, 0:512], u_sb[:, 0:128], h[:, exs], start=True, stop=False)
                    nc.tensor.matmul(ps[:, 0:512], m_sb[:, 0:128], xgh[:, exs], start=False, stop=False)
                    nc.tensor.matmul(ps[:, 0:512], ones_sb[:],
                                     nm_sb[0:1, (t % NMCH) * bc + gi * 512:(t % NMCH) * bc + (gi + 1) * 512],
                                     start=False, stop=not with_czr)
                    if with_czr:
                        nc.tensor.matmul(ps[:, 0:512], czr_sb[:, 0:128], onesbc_sb[:, exs],
                                         start=False, stop=True)

                def r_mms():
                    nc.tensor.matmul(ps[:, 512:1024], u_sb[:, 128:256], h[:, exs], start=True, stop=False)
                    nc.tensor.matmul(ps[:, 512:1024], m_sb[:, 128:256], xgh[:, exs],
                                     start=False, stop=not with_czr)
                    if with_czr:
                        nc.tensor.matmul(ps[:, 512:1024], czr_sb[:, 128:256], onesbc_sb[:, exs],
                                         start=False, stop=True)

                def g_mm():
                    nc.tensor.matmul(pG, u_sb[:, 256:384], h[:, exs], start=True, stop=True)

                if R_FIRST:
                    r_mms(); g_mm(); z_mms()
                else:
                    z_mms(); r_mms(); g_mm()
                if SIG_SPLIT:
                    nc.scalar.activation(zr[:, gi * 1024 + 512:(gi + 1) * 1024],
                                         ps[:, 512:1024], AF.Sigmoid)
                    nc.scalar.activation(zr[:, gi * 1024:gi * 1024 + 512],
                                         ps[:, 0:512], AF.Sigmoid)
                else:
                    nc.scalar.activation(zr[:, gi * 1024:(gi + 1) * 1024], ps, AF.Sigmoid)
                nc.vector.scalar_tensor_tensor(
                    t1[:, exs], pG, b1h_sb[:],
                    zr[:, gi * 1024 + 512:(gi + 1) * 1024], OP.add, OP.mult,
                )
            t2 = tp.tile([128, bc], F16, tag="t2")
            hh = tp.tile([128, bc], F16, tag="hh")
            dd = tp.tile([128, bc], F16, tag="dd")
            m1 = tp.tile([128, bc], F16, tag="m1")
            def blend(pi):
                sl = slice(pi * pw, (pi + 1) * pw)
                nc.vector.tensor_sub(dd[:, sl], h[:, sl], hh[:, sl])
                zv = zr[:, pi * gperp * 1024:(pi + 1) * gperp * 1024] \
                    .rearrange("p (g c) -> p g c", g=gperp)[:, :, 0:512]
                dv = dd[:, sl].rearrange("p (g c) -> p g c", g=gperp)
                mv = m1[:, sl].rearrange("p (g c) -> p g c", g=gperp)
                nc.vector.tensor_mul(mv, zv, dv)
                nc.vector.tensor_add(hnew[:, sl], m1[:, sl], hh[:, sl])

            if TANH_MERGE and npairs % 2 == 0:
                for pi in range(0, npairs, 2):
                    sla = slice(pi * pw, (pi + 1) * pw)
                    slb = slice((pi + 1) * pw, (pi + 2) * pw)
                    nc.vector.tensor_add(t2[:, sla], t1[:, sla], xgh[:, sla])
                    nc.vector.tensor_add(t2[:, slb], t1[:, slb], xgh[:, slb])
                    sl2 = slice(pi * pw, (pi + 2) * pw)
                    nc.scalar.activation(hh[:, sl2], t2[:, sl2], AF.Tanh)
                    blend(pi)
                    blend(pi + 1)
            else:
                for pi in range(npairs):
                    sl = slice(pi * pw, (pi + 1) * pw)
                    nc.vector.tensor_add(t2[:, sl], t1[:, sl], xgh[:, sl])
                    nc.scalar.activation(hh[:, sl], t2[:, sl], AF.Tanh)
                    blend(pi)
            h = hnew

        out_sb = hd.tile([128, (bc // 128) * C], F32, tag="out")
        et_all = hd.tile([128, (bc // 128) * C], F32, tag="eta")
        ss_all = hd.tile([128, (bc // 128)], F32, tag="ssa")
        for hg in range(bc // 512):
            if HEAD_SHARE_PS:
                psd_t = pzr.tile([128, 1024], F32, tag="ps")
                psd = psd_t[:, 0:512]
            else:
                psd_t = ph.tile([128, 512], F32, tag="hps")
                psd = psd_t[:]
            nc.tensor.matmul(psd, w1_sb[:], h[:, hg * 512:(hg + 1) * 512], start=True, stop=True)
            sg = hd.tile([128, 512], F16, tag="sg")
            nc.scalar.activation(sg[:], psd, AF.Sigmoid, bias=b1c_sb[:])
            dt = hd.tile([128, 512], F16, tag="dt")
            # swish(d) = d * sigmoid(d), d = psd + b1
            nc.vector.scalar_tensor_tensor(dt[:], psd, b1c_sb[:], sg[:], OP.add, OP.mult)
            for sub in range(4):
                if HEAD_SHARE_PS:
                    psl_t = pzr.tile([128, 1024], F32, tag="ps")
                    psl = psl_t[:, 0:C]
                else:
                    psl_t = ph.tile([128, C], F32, tag="hps")
                    psl = psl_t[:]
                nc.tensor.matmul(psl, dt[:, sub * 128:(sub + 1) * 128], wout_sb[:], start=True, stop=False)
                nc.tensor.matmul(psl, ones_sb[:], bout_sb[:], start=False, stop=True)
                i = hg * 4 + sub
                nc.scalar.activation(et_all[:, i * C:(i + 1) * C], psl, AF.Exp,
                                     accum_out=ss_all[:, i:i + 1])
        rc_all = hd.tile([128, (bc // 128)], F32, tag="rc")
        nc.vector.reciprocal(rc_all[:], ss_all[:])
        for i in range(bc // 128):
            nc.vector.tensor_scalar_mul(out_sb[:, i * C:(i + 1) * C],
                                        et_all[:, i * C:(i + 1) * C], rc_all[:, i:i + 1])
        nc.sync.dma_start(outp, out_sb[:])

    nc.compile()
    return nc


def prep_tables(emb, W, U, b, W1, b1, Wout, bout):
    """Host-side weight preprocessing -> (shared input dict, with_czr flag)."""
    f16 = np.float16
    emb = np.asarray(emb, np.float64)
    W = np.asarray(W, np.float64)
    b = np.asarray(b, np.float64)
    Wh = W[:, 256:384]
    gtab = (emb @ Wh + b[0, 256:384]).astype(f16)          # [V, 128]
    Minv = np.linalg.pinv(Wh)                               # [128, 32]
    M = (Minv @ W[:, 0:256]).astype(f16)                    # [128, 256]
    # xg_zr = (xgh - b0_h) @ M + b0_zr  (+ b1_zr folded with it)
    c = (-b[0, 256:384] @ Minv @ W[:, 0:256] + b[0, 0:256] + b[1, 0:256])
    with_czr = bool(np.any(np.abs(c) > 1e-12))
    shared = {
        "gtab": gtab,
        "uzrh": np.asarray(U, np.float32).astype(f16),
        "mzr": M,
        "b1h": np.asarray(b[1, 256:384], np.float32).reshape(128, 1).copy(),
        "w1": np.asarray(W1, np.float32).astype(f16),
        "b1c": np.asarray(b1, np.float32).reshape(128, 1).copy(),
        "wout": np.asarray(Wout, np.float32).astype(f16),
        "boutw": np.asarray(bout, np.float32).reshape(1, C).astype(f16),
    }
    if with_czr:
        shared["czr"] = c.reshape(1, 256).astype(f16)
    return shared, with_czr


def prep_idx(tokens_core, nt):
    """tokens_core [bc, nt] int -> wrapped idx tensor [128, nt*bc/16] int16."""
    bc = tokens_core.shape[0]
    tk = np.ascontiguousarray(tokens_core.astype(np.int16))
    w = tk.T.reshape(nt, bc // 16, 16).transpose(0, 2, 1)   # [t, r, c16]
    w = np.tile(w, (1, 8, 1))
    return np.ascontiguousarray(w.transpose(1, 0, 2).reshape(128, nt * bc // 16))


def prep_notm(tokens_core, nt):
    """[1, nt*bc] f16: BIGM where token==0 else 0 (z-gate freeze logit)."""
    return np.ascontiguousarray(
        ((tokens_core.T == 0).astype(np.float16) * np.float16(BIGM)).reshape(1, -1))


def assemble_out(res_core, bc=BC):
    """[128, (bc/128)*3] f32 device output -> [bc, 3]."""
    return np.ascontiguousarray(
        res_core.reshape(128, bc // 128, C).transpose(1, 0, 2).reshape(bc, C)
    )


_NC_CACHE = {}


def kernel(tokens, emb, W, U, b, W1, b1, Wout, bout):
    tokens = np.asarray(tokens)
    shared, with_czr = prep_tables(emb, W, U, b, W1, b1, Wout, bout)
    key = (BC, T, with_czr)
    if key not in _NC_CACHE:
        _NC_CACHE[key] = build_nc(BC, T, with_czr)
    nc = _NC_CACHE[key]
    in_maps = []
    for c in range(NCORES):
        m = dict(shared)
        tc = tokens[c * BC:(c + 1) * BC]
        m["idxw"] = prep_idx(tc, T)
        m["notm"] = prep_notm(tc, T)
        in_maps.append(m)
    res = run_bass_kernel_spmd(nc, in_maps, core_ids=list(range(NCORES)))
    out = np.concatenate([assemble_out(res.results[c]["outp"], BC) for c in range(NCORES)], axis=0)
    return out.astype(np.float32)



# revision 4
# speedup vs baseline: 1.0029x; 1.0029x over previous
"""Trainium2 Bass kernel for nn_AttentionRNN (embedding + masked GRU + MLP head + softmax).

Strategy (pure data parallelism over 8 NeuronCores, 2048 examples/core):

Layout: everything transposed — state h kept as hT [H=128 partitions, examples
on free dim], so the GRU recurrence is closed under the layout (no per-step
transposes). Per time step t, per 512-example group (psum tile [128, 1536] f32):

  psum[:, 0:512]    = U_z.T @ hT + M_z.T @ xghT + 1s.T @ notm_t   (z preact)
  psum[:, 512:1024] = U_r.T @ hT + M_r.T @ xghT                   (r preact)
  psum[:, 1024:1536]= U_h.T @ hT                                  (rec_h)
  z|r  = sigmoid(psum[:, 0:1024])                 (one ACT call, reads PSUM)
  t1   = (rec_h + b1_h) * r                       (fused DVE scalar_tensor_tensor)
  t2   = t1 + xgh
  hh   = tanh(t2)
  h'   = z*(h - hh) + hh                          (3 DVE tensor_tensor ops)

The only gather is xghT: dma_gather (transpose mode) from a host-precomputed
fp16 table gtab[V, 128] = emb @ W[:, 256:384] + b0_h  (256B rows).  The z/r
input projections are reconstructed algebraically instead of gathered:
  x = (xgh - b0_h) @ pinv(W_h)  (exact: xgh lies in W_h's 32-dim row space)
  xg_zr = x @ W_zr = xgh @ M + c,  M = pinv(W_h) @ W_zr  (host-precomputed)
so no second gather is needed.  dma_gather is HW-limited to ~896 idxs/call
(1024+ kills the Pool engine), so each step gathers in (896, 896, 256) splits.

Mask (token==0 freezes state): notmT[t, i] = 100 if token==0 else 0, shipped
from host; a K=1 matmul adds it to the z preactivation => z = sigmoid(.+100) = 1
exactly => h' = h.  Biases: b0_h folded into gtab; b1_h via the STT scalar;
b0/b1_zr (+ the -b0_h@M correction) via K=1 matmuls only when nonzero.

Head: dT = swish(W1.T @ hT + b1); logits per 128-example tile with examples on
partitions (lhsT = dT slice); softmax along free dim (C=3).
"""

import numpy as np
from contextlib import ExitStack

import concourse.mybir as mybir
import concourse.tile as tile
from concourse import bacc
from concourse.bass_utils import run_bass_kernel_spmd

B, T, E, H, V, D, C = 16384, 128, 32, 128, 30001, 128, 3
NCORES = 8
BC = B // NCORES
BIGM = 100.0
NIDX = 896             # max idxs per dma_gather call (HW-probed ucode limit:
                       # 896 works, 1024+ crashes the Pool engine)
USE_SPLIT = True       # split zr/g psum tiles vs one 3-bank tile
G_BUFS = 6
H_BUFS = 3
Z_BUFS = 3
TMP_BUFS = 3
NM_CH = 4
PW = 512
SIG_SPLIT = False
TANH_MERGE = True
HEAD_SHARE_PS = True
R_FIRST = False
SKIP_GATHER = False
GSPLIT_OVERRIDE = None
PSZR_BUFS = 3          # psum pool depths (8 banks total: 2*PSZR + PSG + PH)
PSG_BUFS = 2
PH_BUFS = 2

F16 = mybir.dt.float16
F32 = mybir.dt.float32
I16 = mybir.dt.int16
AF = mybir.ActivationFunctionType
OP = mybir.AluOpType
AX = mybir.AxisListType


def build_nc(bc=BC, nt=T, with_czr=False):
    """Build + compile the per-core Bass program. bc = examples per core."""
    assert bc % 512 == 0
    ng = bc // 512            # 512-example groups per step
    pw = min(PW, bc)          # width of the wide DVE ops
    npairs = bc // pw
    gperp = pw // 512
    gsplit = []
    off = 0
    while off < bc:
        n = min(NIDX, bc - off)
        gsplit.append((off, n))
        off += n
    if GSPLIT_OVERRIDE:
        gsplit = GSPLIT_OVERRIDE

    nc = bacc.Bacc("TRN2", target_bir_lowering=False, debug=False)
    gtab = nc.dram_tensor("gtab", [V, 128], F16, kind="ExternalInput").ap()
    idxw = nc.dram_tensor("idxw", [128, nt * bc // 16], I16, kind="ExternalInput").ap()
    uzrh = nc.dram_tensor("uzrh", [128, 384], F16, kind="ExternalInput").ap()
    mzr = nc.dram_tensor("mzr", [128, 256], F16, kind="ExternalInput").ap()
    notm = nc.dram_tensor("notm", [1, nt * bc], F16, kind="ExternalInput").ap()
    b1h = nc.dram_tensor("b1h", [128, 1], F32, kind="ExternalInput").ap()
    w1 = nc.dram_tensor("w1", [128, 128], F16, kind="ExternalInput").ap()
    b1c = nc.dram_tensor("b1c", [128, 1], F32, kind="ExternalInput").ap()
    wout = nc.dram_tensor("wout", [128, C], F16, kind="ExternalInput").ap()
    boutw = nc.dram_tensor("boutw", [1, C], F16, kind="ExternalInput").ap()
    if with_czr:
        czr = nc.dram_tensor("czr", [1, 256], F16, kind="ExternalInput").ap()
    outp = nc.dram_tensor("outp", [128, (bc // 128) * C], F32, kind="ExternalOutput").ap()

    with tile.TileContext(nc) as tc, ExitStack() as ctx:
        wp = ctx.enter_context(tc.tile_pool(name="w", bufs=1))
        ip = ctx.enter_context(tc.tile_pool(name="idx", bufs=1))
        gp = ctx.enter_context(tc.tile_pool(name="g", bufs=G_BUFS))
        hp = ctx.enter_context(tc.tile_pool(name="h", bufs=H_BUFS))
        zp = ctx.enter_context(tc.tile_pool(name="zr", bufs=Z_BUFS))
        tp = ctx.enter_context(tc.tile_pool(name="tmp", bufs=TMP_BUFS))
        pzr = ctx.enter_context(tc.tile_pool(name="pszr", bufs=PSZR_BUFS, space="PSUM"))
        pg = ctx.enter_context(tc.tile_pool(name="psg", bufs=PSG_BUFS, space="PSUM"))
        hd = ctx.enter_context(tc.tile_pool(name="hd", bufs=2))
        ph = ctx.enter_context(tc.tile_pool(name="ph", bufs=PH_BUFS, space="PSUM"))

        u_sb = wp.tile([128, 384], F16, tag="u")
        nc.sync.dma_start(u_sb[:], uzrh)
        m_sb = wp.tile([128, 256], F16, tag="mzr")
        nc.sync.dma_start(m_sb[:], mzr)
        b1h_sb = wp.tile([128, 1], F32, tag="b1h")
        nc.sync.dma_start(b1h_sb[:], b1h)
        w1_sb = wp.tile([128, 128], F16, tag="w1")
        nc.sync.dma_start(w1_sb[:], w1)
        b1c_sb = wp.tile([128, 1], F32, tag="b1c")
        nc.sync.dma_start(b1c_sb[:], b1c)
        wout_sb = wp.tile([128, C], F16, tag="wo")
        nc.sync.dma_start(wout_sb[:], wout)
        bout_sb = wp.tile([1, C], F16, tag="bo")
        nc.sync.dma_start(bout_sb[:], boutw)
        ones_sb = wp.tile([1, 128], F16, tag="ones")
        nc.vector.memset(ones_sb[:], 1.0)
        # Pin the ACT table set that contains BOTH Sigmoid and Tanh so the
        # auto-placement pass doesn't ping-pong table loads every step
        # (~1.3us per load on the ACT critical path).
        from concourse.hw_specs import get_activation_tables
        _tabs = get_activation_tables(nc.m.arch)
        _setid = next(i for i, (nm2, fs) in enumerate(_tabs.items())
                      if AF.Sigmoid in fs and AF.Tanh in fs)
        nc.scalar.add_instruction(mybir.InstLoadActFuncSet(
            name=nc.get_next_instruction_name(), ins=[], outs=[],
            act_func_set_id=_setid))
        if with_czr:
            czr_sb = wp.tile([1, 256], F16, tag="czr")
            nc.sync.dma_start(czr_sb[:], czr)
            onesbc_sb = wp.tile([1, bc], F16, tag="onesbc")
            nc.vector.memset(onesbc_sb[:], 1.0)
        idx_sb = ip.tile([128, nt * bc // 16], I16, tag="idx")
        nc.sync.dma_start(idx_sb[:], idxw)

        nmp = ctx.enter_context(tc.tile_pool(name="nm", bufs=2))
        NMCH = NM_CH      # timesteps of notm per staged chunk
        h = hp.tile([128, bc], F16, tag="h")
        nc.vector.memset(h[:], 0.0)

        nm_sb = None
        for t in range(nt):
            if t % NMCH == 0:
                nm_sb = nmp.tile([1, NMCH * bc], F16, tag="nm")
                nc.sync.dma_start(nm_sb[:], notm[:, t * bc:(t + NMCH) * bc])
            g = gp.tile([128, 1, bc], F16, tag="g")
            if SKIP_GATHER:       # timing-probe flag: cheap gpsimd fill instead
                nc.gpsimd.memset(g[:], 0.01)
            else:
                for off, n in gsplit:
                    nc.gpsimd.dma_gather(
                        g[:, :, off:off + n], gtab,
                        idx_sb[:, (t * bc + off) // 16:(t * bc + off + n) // 16],
                        n, n, 128, transpose=True,
                    )
            xgh = g[:, 0, :]
            zr = zp.tile([128, 2 * bc], F16, tag="zr")
            t1 = tp.tile([128, bc], F16, tag="t1")
            hnew = hp.tile([128, bc], F16, tag="h")
            for gi in range(ng):
                if USE_SPLIT:
                    ps_t = pzr.tile([128, 1024], F32, tag="ps")
                    pG_t = pg.tile([128, 512], F32, tag="pg")
                    ps = ps_t[:]
                    pG = pG_t[:]
                else:
                    both = pzr.tile([128, 1536], F32, tag="ps")
                    ps = both[:, 0:1024]
                    pG = both[:, 1024:1536]
                exs = slice(gi * 512, (gi + 1) * 512)
                def z_mms():
                    nc.tensor.matmul(ps[:, 0:512], u_sb[:, 0:128], h[:, exs], start=True, stop=False)
                    nc.tensor.matmul(ps[:, 0:512], m_sb[:, 0:128], xgh[:, exs], start=False, stop=False)
                    nc.tensor.matmul(ps[:, 0:512], ones_sb[:],
                                     nm_sb[0:1, (t % NMCH) * bc + gi * 512:(t % NMCH) * bc + (gi + 1) * 512],
                                     start=False, stop=not with_czr)
                    if with_czr:
                        nc.tensor.matmul(ps[:, 0:512], czr_sb[:, 0:128], onesbc_sb[:, exs],
                                         start=False, stop=True)

                def r_mms():
                    nc.tensor.matmul(ps[:, 512:1024], u_sb[:, 128:256], h[:, exs], start=True, stop=False)
                    nc.tensor.matmul(ps[:, 512:1024], m_sb[:, 128:256], xgh[:, exs],
                                     start=False, stop=not with_czr)
                    if with_czr:
                        nc.tensor.matmul(ps[:, 512:1024], czr_sb[:, 128:256], onesbc_sb[:, exs],
                                         start=False, stop=True)

                def g_mm():
                    nc.tensor.matmul(pG, u_sb[:, 256:384], h[:, exs], start=True, stop=True)

                if R_FIRST:
                    r_mms(); g_mm(); z_mms()
                else:
                    z_mms(); r_mms(); g_mm()
                if SIG_SPLIT:
                    nc.scalar.activation(zr[:, gi * 1024 + 512:(gi + 1) * 1024],
                                         ps[:, 512:1024], AF.Sigmoid)
                    nc.scalar.activation(zr[:, gi * 1024:gi * 1024 + 512],
                                         ps[:, 0:512], AF.Sigmoid)
                else:
                    nc.scalar.activation(zr[:, gi * 1024:(gi + 1) * 1024], ps, AF.Sigmoid)
                nc.vector.scalar_tensor_tensor(
                    t1[:, exs], pG, b1h_sb[:],
                    zr[:, gi * 1024 + 512:(gi + 1) * 1024], OP.add, OP.mult,
                )
            t2 = tp.tile([128, bc], F16, tag="t2")
            hh = tp.tile([128, bc], F16, tag="hh")
            dd = tp.tile([128, bc], F16, tag="dd")
            m1 = tp.tile([128, bc], F16, tag="m1")
            def blend(pi):
                sl = slice(pi * pw, (pi + 1) * pw)
                nc.vector.tensor_sub(dd[:, sl], h[:, sl], hh[:, sl])
                zv = zr[:, pi * gperp * 1024:(pi + 1) * gperp * 1024] \
                    .rearrange("p (g c) -> p g c", g=gperp)[:, :, 0:512]
                dv = dd[:, sl].rearrange("p (g c) -> p g c", g=gperp)
                mv = m1[:, sl].rearrange("p (g c) -> p g c", g=gperp)
                nc.vector.tensor_mul(mv, zv, dv)
                nc.vector.tensor_add(hnew[:, sl], m1[:, sl], hh[:, sl])

            if TANH_MERGE and npairs % 2 == 0:
                for pi in range(0, npairs, 2):
                    sla = slice(pi * pw, (pi + 1) * pw)
                    slb = slice((pi + 1) * pw, (pi + 2) * pw)
                    nc.vector.tensor_add(t2[:, sla], t1[:, sla], xgh[:, sla])
                    nc.vector.tensor_add(t2[:, slb], t1[:, slb], xgh[:, slb])
                    sl2 = slice(pi * pw, (pi + 2) * pw)
                    nc.scalar.activation(hh[:, sl2], t2[:, sl2], AF.Tanh)
                    blend(pi)
                    blend(pi + 1)
            else:
                for pi in range(npairs):
                    sl = slice(pi * pw, (pi + 1) * pw)
                    nc.vector.tensor_add(t2[:, sl], t1[:, sl], xgh[:, sl])
                    nc.scalar.activation(hh[:, sl], t2[:, sl], AF.Tanh)
                    blend(pi)
            h = hnew

        out_sb = hd.tile([128, (bc // 128) * C], F32, tag="out")
        et_all = hd.tile([128, (bc // 128) * C], F32, tag="eta")
        ss_all = hd.tile([128, (bc // 128)], F32, tag="ssa")
        for hg in range(bc // 512):
            if HEAD_SHARE_PS:
                psd_t = pzr.tile([128, 1024], F32, tag="ps")
                psd = psd_t[:, 0:512]
            else:
                psd_t = ph.tile([128, 512], F32, tag="hps")
                psd = psd_t[:]
            nc.tensor.matmul(psd, w1_sb[:], h[:, hg * 512:(hg + 1) * 512], start=True, stop=True)
            sg = hd.tile([128, 512], F16, tag="sg")
            nc.scalar.activation(sg[:], psd, AF.Sigmoid, bias=b1c_sb[:])
            dt = hd.tile([128, 512], F16, tag="dt")
            # swish(d) = d * sigmoid(d), d = psd + b1
            nc.vector.scalar_tensor_tensor(dt[:], psd, b1c_sb[:], sg[:], OP.add, OP.mult)
            for sub in range(4):
                if HEAD_SHARE_PS:
                    psl_t = pzr.tile([128, 1024], F32, tag="ps")
                    psl = psl_t[:, 0:C]
                else:
                    psl_t = ph.tile([128, C], F32, tag="hps")
                    psl = psl_t[:]
                nc.tensor.matmul(psl, dt[:, sub * 128:(sub + 1) * 128], wout_sb[:], start=True, stop=False)
                nc.tensor.matmul(psl, ones_sb[:], bout_sb[:], start=False, stop=True)
                i = hg * 4 + sub
                nc.scalar.activation(et_all[:, i * C:(i + 1) * C], psl, AF.Exp,
                                     accum_out=ss_all[:, i:i + 1])
        rc_all = hd.tile([128, (bc // 128)], F32, tag="rc")
        nc.vector.reciprocal(rc_all[:], ss_all[:])
        for i in range(bc // 128):
            nc.vector.tensor_scalar_mul(out_sb[:, i * C:(i + 1) * C],
                                        et_all[:, i * C:(i + 1) * C], rc_all[:, i:i + 1])
        nc.sync.dma_start(outp, out_sb[:])

    nc.compile()
    return nc


def prep_tables(emb, W, U, b, W1, b1, Wout, bout):
    """Host-side weight preprocessing -> (shared input dict, with_czr flag)."""
    f16 = np.float16
    emb = np.asarray(emb, np.float64)
    W = np.asarray(W, np.float64)
    b = np.asarray(b, np.float64)
    Wh = W[:, 256:384]
    gtab = (emb @ Wh + b[0, 256:384]).astype(f16)          # [V, 128]
    Minv = np.linalg.pinv(Wh)                               # [128, 32]
    M = (Minv @ W[:, 0:256]).astype(f16)                    # [128, 256]
    # xg_zr = (xgh - b0_h) @ M + b0_zr  (+ b1_zr folded with it)
    c = (-b[0, 256:384] @ Minv @ W[:, 0:256] + b[0, 0:256] + b[1, 0:256])
    with_czr = bool(np.any(np.abs(c) > 1e-12))
    shared = {
        "gtab": gtab,
        "uzrh": np.asarray(U, np.float32).astype(f16),
        "mzr": M,
        "b1h": np.asarray(b[1, 256:384], np.float32).reshape(128, 1).copy(),
        "w1": np.asarray(W1, np.float32).astype(f16),
        "b1c": np.asarray(b1, np.float32).reshape(128, 1).copy(),
        "wout": np.asarray(Wout, np.float32).astype(f16),
        "boutw": np.asarray(bout, np.float32).reshape(1, C).astype(f16),
    }
    if with_czr:
        shared["czr"] = c.reshape(1, 256).astype(f16)
    return shared, with_czr


def prep_idx(tokens_core, nt):
    """tokens_core [bc, nt] int -> wrapped idx tensor [128, nt*bc/16] int16."""
    bc = tokens_core.shape[0]
    tk = np.ascontiguousarray(tokens_core.astype(np.int16))
    w = tk.T.reshape(nt, bc // 16, 16).transpose(0, 2, 1)   # [t, r, c16]
    w = np.tile(w, (1, 8, 1))
    return np.ascontiguousarray(w.transpose(1, 0, 2).reshape(128, nt * bc // 16))


def prep_notm(tokens_core, nt):
    """[1, nt*bc] f16: BIGM where token==0 else 0 (z-gate freeze logit)."""
    return np.ascontiguousarray(
        ((tokens_core.T == 0).astype(np.float16) * np.float16(BIGM)).reshape(1, -1))


def assemble_out(res_core, bc=BC):
    """[128, (bc/128)*3] f32 device output -> [bc, 3]."""
    return np.ascontiguousarray(
        res_core.reshape(128, bc // 128, C).transpose(1, 0, 2).reshape(bc, C)
    )


_NC_CACHE = {}


def kernel(tokens, emb, W, U, b, W1, b1, Wout, bout):
    tokens = np.asarray(tokens)
    shared, with_czr = prep_tables(emb, W, U, b, W1, b1, Wout, bout)
    key = (BC, T, with_czr)
    if key not in _NC_CACHE:
        _NC_CACHE[key] = build_nc(BC, T, with_czr)
    nc = _NC_CACHE[key]
    in_maps = []
    for c in range(NCORES):
        m = dict(shared)
        tc = tokens[c * BC:(c + 1) * BC]
        m["idxw"] = prep_idx(tc, T)
        m["notm"] = prep_notm(tc, T)
        in_maps.append(m)
    res = run_bass_kernel_spmd(nc, in_maps, core_ids=list(range(NCORES)))
    out = np.concatenate([assemble_out(res.results[c]["outp"], BC) for c in range(NCORES)], axis=0)
    return out.astype(np.float32)



# revision 7
# speedup vs baseline: 1.5943x; 1.5897x over previous
"""Trainium2 Bass kernel for nn_AttentionRNN (embedding + masked GRU + MLP head + softmax).

Strategy (pure data parallelism over 8 NeuronCores, 2048 examples/core):

Layout: everything transposed — state h kept as hT [H=128 partitions, examples
on free dim], so the GRU recurrence is closed under the layout (no per-step
transposes). Per time step t, per 512-example group (psum tile [128, 1536] f32):

  psum[:, 0:512]    = U_z.T @ hT + M_z.T @ xghT + 1s.T @ notm_t   (z preact)
  psum[:, 512:1024] = U_r.T @ hT + M_r.T @ xghT                   (r preact)
  psum[:, 1024:1536]= U_h.T @ hT                                  (rec_h)
  z|r  = sigmoid(psum[:, 0:1024])                 (one ACT call, reads PSUM)
  t1   = (rec_h + b1_h) * r                       (fused DVE scalar_tensor_tensor)
  t2   = t1 + xgh
  hh   = tanh(t2)
  h'   = z*(h - hh) + hh                          (3 DVE tensor_tensor ops)

The only gather is xghT: dma_gather (transpose mode) from a host-precomputed
fp16 table gtab[V, 128] = emb @ W[:, 256:384] + b0_h  (256B rows).  The z/r
input projections are reconstructed algebraically instead of gathered:
  x = (xgh - b0_h) @ pinv(W_h)  (exact: xgh lies in W_h's 32-dim row space)
  xg_zr = x @ W_zr = xgh @ M + c,  M = pinv(W_h) @ W_zr  (host-precomputed)
so no second gather is needed.  dma_gather is HW-limited to ~896 idxs/call
(1024+ kills the Pool engine), so each step gathers in (896, 896, 256) splits.

Mask (token==0 freezes state): notmT[t, i] = 100 if token==0 else 0, shipped
from host; a K=1 matmul adds it to the z preactivation => z = sigmoid(.+100) = 1
exactly => h' = h.  Biases: b0_h folded into gtab; b1_h via the STT scalar;
b0/b1_zr (+ the -b0_h@M correction) via K=1 matmuls only when nonzero.

Head: dT = swish(W1.T @ hT + b1); logits per 128-example tile with examples on
partitions (lhsT = dT slice); softmax along free dim (C=3).
"""

import numpy as np
from contextlib import ExitStack

import concourse.mybir as mybir
import concourse.tile as tile
from concourse import bacc
from concourse.bass_utils import run_bass_kernel_spmd

B, T, E, H, V, D, C = 16384, 128, 32, 128, 30001, 128, 3
NCORES = 8
BC = B // NCORES
BIGM = 100.0
NIDX = 896             # max idxs per dma_gather call (HW-probed ucode limit:
                       # 896 works, 1024+ crashes the Pool engine)
USE_SPLIT = True       # split zr/g psum tiles vs one 3-bank tile
G_BUFS = 6
H_BUFS = 3
Z_BUFS = 3
TMP_BUFS = 3
NM_CH = 4
PW = 512
SIG_SPLIT = False
TANH_MERGE = True
HEAD_SHARE_PS = True
R_FIRST = False
SKIP_GATHER = False
GSPLIT_OVERRIDE = None
PSZR_BUFS = 3          # psum pool depths (8 banks total: 2*PSZR + PSG + PH)
PSG_BUFS = 2
PH_BUFS = 2

F16 = mybir.dt.float16
F32 = mybir.dt.float32
I16 = mybir.dt.int16
AF = mybir.ActivationFunctionType
OP = mybir.AluOpType
AX = mybir.AxisListType


def build_nc(bc=BC, nt=T, with_czr=False):
    """Build + compile the per-core Bass program. bc = examples per core."""
    assert bc % 512 == 0
    ng = bc // 512            # 512-example groups per step
    pw = min(PW, bc)          # width of the wide DVE ops
    npairs = bc // pw
    gperp = pw // 512
    gsplit = []
    off = 0
    while off < bc:
        n = min(NIDX, bc - off)
        gsplit.append((off, n))
        off += n
    if GSPLIT_OVERRIDE:
        gsplit = GSPLIT_OVERRIDE

    nc = bacc.Bacc("TRN2", target_bir_lowering=False, debug=False,
                   num_swdge_queues=4)
    gtab = nc.dram_tensor("gtab", [V, 128], F16, kind="ExternalInput").ap()
    idxw = nc.dram_tensor("idxw", [128, nt * bc // 16], I16, kind="ExternalInput").ap()
    uzrh = nc.dram_tensor("uzrh", [128, 384], F16, kind="ExternalInput").ap()
    mzr = nc.dram_tensor("mzr", [128, 256], F16, kind="ExternalInput").ap()
    notm = nc.dram_tensor("notm", [1, nt * bc], F16, kind="ExternalInput").ap()
    b1h = nc.dram_tensor("b1h", [128, 1], F32, kind="ExternalInput").ap()
    w1 = nc.dram_tensor("w1", [128, 128], F16, kind="ExternalInput").ap()
    b1c = nc.dram_tensor("b1c", [128, 1], F32, kind="ExternalInput").ap()
    wout = nc.dram_tensor("wout", [128, C], F16, kind="ExternalInput").ap()
    boutw = nc.dram_tensor("boutw", [1, C], F16, kind="ExternalInput").ap()
    if with_czr:
        czr = nc.dram_tensor("czr", [1, 256], F16, kind="ExternalInput").ap()
    outp = nc.dram_tensor("outp", [128, (bc // 128) * C], F32, kind="ExternalOutput").ap()

    with tile.TileContext(nc) as tc, ExitStack() as ctx:
        wp = ctx.enter_context(tc.tile_pool(name="w", bufs=1))
        ip = ctx.enter_context(tc.tile_pool(name="idx", bufs=1))
        gp = ctx.enter_context(tc.tile_pool(name="g", bufs=G_BUFS))
        hp = ctx.enter_context(tc.tile_pool(name="h", bufs=H_BUFS))
        zp = ctx.enter_context(tc.tile_pool(name="zr", bufs=Z_BUFS))
        tp = ctx.enter_context(tc.tile_pool(name="tmp", bufs=TMP_BUFS))
        pzr = ctx.enter_context(tc.tile_pool(name="pszr", bufs=PSZR_BUFS, space="PSUM"))
        pg = ctx.enter_context(tc.tile_pool(name="psg", bufs=PSG_BUFS, space="PSUM"))
        hd = ctx.enter_context(tc.tile_pool(name="hd", bufs=2))
        ph = ctx.enter_context(tc.tile_pool(name="ph", bufs=PH_BUFS, space="PSUM"))

        u_sb = wp.tile([128, 384], F16, tag="u")
        nc.sync.dma_start(u_sb[:], uzrh)
        m_sb = wp.tile([128, 256], F16, tag="mzr")
        nc.sync.dma_start(m_sb[:], mzr)
        b1h_sb = wp.tile([128, 1], F32, tag="b1h")
        nc.sync.dma_start(b1h_sb[:], b1h)
        w1_sb = wp.tile([128, 128], F16, tag="w1")
        nc.sync.dma_start(w1_sb[:], w1)
        b1c_sb = wp.tile([128, 1], F32, tag="b1c")
        nc.sync.dma_start(b1c_sb[:], b1c)
        wout_sb = wp.tile([128, C], F16, tag="wo")
        nc.sync.dma_start(wout_sb[:], wout)
        bout_sb = wp.tile([1, C], F16, tag="bo")
        nc.sync.dma_start(bout_sb[:], boutw)
        ones_sb = wp.tile([1, 128], F16, tag="ones")
        nc.vector.memset(ones_sb[:], 1.0)
        # Pin the ACT table set that contains BOTH Sigmoid and Tanh so the
        # auto-placement pass doesn't ping-pong table loads every step
        # (~1.3us per load on the ACT critical path).
        from concourse.hw_specs import get_activation_tables
        _tabs = get_activation_tables(nc.m.arch)
        _setid = next(i for i, (nm2, fs) in enumerate(_tabs.items())
                      if AF.Sigmoid in fs and AF.Tanh in fs)
        nc.scalar.add_instruction(mybir.InstLoadActFuncSet(
            name=nc.get_next_instruction_name(), ins=[], outs=[],
            act_func_set_id=_setid))
        if with_czr:
            czr_sb = wp.tile([1, 256], F16, tag="czr")
            nc.sync.dma_start(czr_sb[:], czr)
            onesbc_sb = wp.tile([1, bc], F16, tag="onesbc")
            nc.vector.memset(onesbc_sb[:], 1.0)
        idx_sb = ip.tile([128, nt * bc // 16], I16, tag="idx")
        nc.sync.dma_start(idx_sb[:], idxw)

        nmp = ctx.enter_context(tc.tile_pool(name="nm", bufs=2))
        NMCH = NM_CH      # timesteps of notm per staged chunk
        h = hp.tile([128, bc], F16, tag="h")
        nc.vector.memset(h[:], 0.0)

        nm_sb = None
        _gq = [0]              # global gather-call counter for queue RR
        for t in range(nt):
            if t % NMCH == 0:
                nm_sb = nmp.tile([1, NMCH * bc], F16, tag="nm")
                nc.sync.dma_start(nm_sb[:], notm[:, t * bc:(t + NMCH) * bc])
            g = gp.tile([128, 1, bc], F16, tag="g")
            if SKIP_GATHER:       # timing-probe flag: cheap gpsimd fill instead
                nc.gpsimd.memset(g[:], 0.01)
            else:
                for off, n in gsplit:
                    nc.gpsimd.dma_gather(
                        g[:, :, off:off + n], gtab,
                        idx_sb[:, (t * bc + off) // 16:(t * bc + off + n) // 16],
                        n, n, 128, transpose=True,
                        queue_num=_gq[0] % 4,
                    )
                    _gq[0] += 1
            xgh = g[:, 0, :]
            zr = zp.tile([128, 2 * bc], F16, tag="zr")
            t1 = tp.tile([128, bc], F16, tag="t1")
            hnew = hp.tile([128, bc], F16, tag="h")
            for gi in range(ng):
                if USE_SPLIT:
                    ps_t = pzr.tile([128, 1024], F32, tag="ps")
                    pG_t = pg.tile([128, 512], F32, tag="pg")
                    ps = ps_t[:]
                    pG = pG_t[:]
                else:
                    both = pzr.tile([128, 1536], F32, tag="ps")
                    ps = both[:, 0:1024]
                    pG = both[:, 1024:1536]
                exs = slice(gi * 512, (gi + 1) * 512)
                def z_mms():
                    nc.tensor.matmul(ps[:, 0:512], u_sb[:, 0:128], h[:, exs], start=True, stop=False)
                    nc.tensor.matmul(ps[:, 0:512], m_sb[:, 0:128], xgh[:, exs], start=False, stop=False)
                    nc.tensor.matmul(ps[:, 0:512], ones_sb[:],
                                     nm_sb[0:1, (t % NMCH) * bc + gi * 512:(t % NMCH) * bc + (gi + 1) * 512],
                                     start=False, stop=not with_czr)
                    if with_czr:
                        nc.tensor.matmul(ps[:, 0:512], czr_sb[:, 0:128], onesbc_sb[:, exs],
                                         start=False, stop=True)

                def r_mms():
                    nc.tensor.matmul(ps[:, 512:1024], u_sb[:, 128:256], h[:, exs], start=True, stop=False)
                    nc.tensor.matmul(ps[:, 512:1024], m_sb[:, 128:256], xgh[:, exs],
                                     start=False, stop=not with_czr)
                    if with_czr:
                        nc.tensor.matmul(ps[:, 512:1024], czr_sb[:, 128:256], onesbc_sb[:, exs],
                                         start=False, stop=True)

                def g_mm():
                    nc.tensor.matmul(pG, u_sb[:, 256:384], h[:, exs], start=True, stop=True)

                if R_FIRST:
                    r_mms(); g_mm(); z_mms()
                else:
                    z_mms(); r_mms(); g_mm()
                if SIG_SPLIT:
                    nc.scalar.activation(zr[:, gi * 1024 + 512:(gi + 1) * 1024],
                                         ps[:, 512:1024], AF.Sigmoid)
                    nc.scalar.activation(zr[:, gi * 1024:gi * 1024 + 512],
                                         ps[:, 0:512], AF.Sigmoid)
                else:
                    nc.scalar.activation(zr[:, gi * 1024:(gi + 1) * 1024], ps, AF.Sigmoid)
                nc.vector.scalar_tensor_tensor(
                    t1[:, exs], pG, b1h_sb[:],
                    zr[:, gi * 1024 + 512:(gi + 1) * 1024], OP.add, OP.mult,
                )
            t2 = tp.tile([128, bc], F16, tag="t2")
            hh = tp.tile([128, bc], F16, tag="hh")
            dd = tp.tile([128, bc], F16, tag="dd")
            m1 = tp.tile([128, bc], F16, tag="m1")
            def blend(pi):
                sl = slice(pi * pw, (pi + 1) * pw)
                nc.vector.tensor_sub(dd[:, sl], h[:, sl], hh[:, sl])
                zv = zr[:, pi * gperp * 1024:(pi + 1) * gperp * 1024] \
                    .rearrange("p (g c) -> p g c", g=gperp)[:, :, 0:512]
                dv = dd[:, sl].rearrange("p (g c) -> p g c", g=gperp)
                mv = m1[:, sl].rearrange("p (g c) -> p g c", g=gperp)
                nc.vector.tensor_mul(mv, zv, dv)
                nc.vector.tensor_add(hnew[:, sl], m1[:, sl], hh[:, sl])

            if TANH_MERGE and npairs % 2 == 0:
                for pi in range(0, npairs, 2):
                    sla = slice(pi * pw, (pi + 1) * pw)
                    slb = slice((pi + 1) * pw, (pi + 2) * pw)
                    nc.vector.tensor_add(t2[:, sla], t1[:, sla], xgh[:, sla])
                    nc.vector.tensor_add(t2[:, slb], t1[:, slb], xgh[:, slb])
                    sl2 = slice(pi * pw, (pi + 2) * pw)
                    nc.scalar.activation(hh[:, sl2], t2[:, sl2], AF.Tanh)
                    blend(pi)
                    blend(pi + 1)
            else:
                for pi in range(npairs):
                    sl = slice(pi * pw, (pi + 1) * pw)
                    nc.vector.tensor_add(t2[:, sl], t1[:, sl], xgh[:, sl])
                    nc.scalar.activation(hh[:, sl], t2[:, sl], AF.Tanh)
                    blend(pi)
            h = hnew

        out_sb = hd.tile([128, (bc // 128) * C], F32, tag="out")
        et_all = hd.tile([128, (bc // 128) * C], F32, tag="eta")
        ss_all = hd.tile([128, (bc // 128)], F32, tag="ssa")
        for hg in range(bc // 512):
            if HEAD_SHARE_PS:
                psd_t = pzr.tile([128, 1024], F32, tag="ps")
                psd = psd_t[:, 0:512]
            else:
                psd_t = ph.tile([128, 512], F32, tag="hps")
                psd = psd_t[:]
            nc.tensor.matmul(psd, w1_sb[:], h[:, hg * 512:(hg + 1) * 512], start=True, stop=True)
            sg = hd.tile([128, 512], F16, tag="sg")
            nc.scalar.activation(sg[:], psd, AF.Sigmoid, bias=b1c_sb[:])
            dt = hd.tile([128, 512], F16, tag="dt")
            # swish(d) = d * sigmoid(d), d = psd + b1
            nc.vector.scalar_tensor_tensor(dt[:], psd, b1c_sb[:], sg[:], OP.add, OP.mult)
            for sub in range(4):
                if HEAD_SHARE_PS:
                    psl_t = pzr.tile([128, 1024], F32, tag="ps")
                    psl = psl_t[:, 0:C]
                else:
                    psl_t = ph.tile([128, C], F32, tag="hps")
                    psl = psl_t[:]
                nc.tensor.matmul(psl, dt[:, sub * 128:(sub + 1) * 128], wout_sb[:], start=True, stop=False)
                nc.tensor.matmul(psl, ones_sb[:], bout_sb[:], start=False, stop=True)
                i = hg * 4 + sub
                nc.scalar.activation(et_all[:, i * C:(i + 1) * C], psl, AF.Exp,
                                     accum_out=ss_all[:, i:i + 1])
        rc_all = hd.tile([128, (bc // 128)], F32, tag="rc")
        nc.vector.reciprocal(rc_all[:], ss_all[:])
        for i in range(bc // 128):
            nc.vector.tensor_scalar_mul(out_sb[:, i * C:(i + 1) * C],
                                        et_all[:, i * C:(i + 1) * C], rc_all[:, i:i + 1])
        nc.sync.dma_start(outp, out_sb[:])

    nc.compile()
    return nc


def prep_tables(emb, W, U, b, W1, b1, Wout, bout):
    """Host-side weight preprocessing -> (shared input dict, with_czr flag)."""
    f16 = np.float16
    emb = np.asarray(emb, np.float64)
    W = np.asarray(W, np.float64)
    b = np.asarray(b, np.float64)
    Wh = W[:, 256:384]
    gtab = (emb @ Wh + b[0, 256:384]).astype(f16)          # [V, 128]
    Minv = np.linalg.pinv(Wh)                               # [128, 32]
    M = (Minv @ W[:, 0:256]).astype(f16)                    # [128, 256]
    # xg_zr = (xgh - b0_h) @ M + b0_zr  (+ b1_zr folded with it)
    c = (-b[0, 256:384] @ Minv @ W[:, 0:256] + b[0, 0:256] + b[1, 0:256])
    with_czr = bool(np.any(np.abs(c) > 1e-12))
    shared = {
        "gtab": gtab,
        "uzrh": np.asarray(U, np.float32).astype(f16),
        "mzr": M,
        "b1h": np.asarray(b[1, 256:384], np.float32).reshape(128, 1).copy(),
        "w1": np.asarray(W1, np.float32).astype(f16),
        "b1c": np.asarray(b1, np.float32).reshape(128, 1).copy(),
        "wout": np.asarray(Wout, np.float32).astype(f16),
        "boutw": np.asarray(bout, np.float32).reshape(1, C).astype(f16),
    }
    if with_czr:
        shared["czr"] = c.reshape(1, 256).astype(f16)
    return shared, with_czr


def prep_idx(tokens_core, nt):
    """tokens_core [bc, nt] int -> wrapped idx tensor [128, nt*bc/16] int16."""
    bc = tokens_core.shape[0]
    tk = np.ascontiguousarray(tokens_core.astype(np.int16))
    w = tk.T.reshape(nt, bc // 16, 16).transpose(0, 2, 1)   # [t, r, c16]
    w = np.tile(w, (1, 8, 1))
    return np.ascontiguousarray(w.transpose(1, 0, 2).reshape(128, nt * bc // 16))


def prep_notm(tokens_core, nt):
    """[1, nt*bc] f16: BIGM where token==0 else 0 (z-gate freeze logit)."""
    return np.ascontiguousarray(
        ((tokens_core.T == 0).astype(np.float16) * np.float16(BIGM)).reshape(1, -1))


def assemble_out(res_core, bc=BC):
    """[128, (bc/128)*3] f32 device output -> [bc, 3]."""
    return np.ascontiguousarray(
        res_core.reshape(128, bc // 128, C).transpose(1, 0, 2).reshape(bc, C)
    )


_NC_CACHE = {}


def kernel(tokens, emb, W, U, b, W1, b1, Wout, bout):
    tokens = np.asarray(tokens)
    shared, with_czr = prep_tables(emb, W, U, b, W1, b1, Wout, bout)
    key = (BC, T, with_czr)
    if key not in _NC_CACHE:
        _NC_CACHE[key] = build_nc(BC, T, with_czr)
    nc = _NC_CACHE[key]
    in_maps = []
    for c in range(NCORES):
        m = dict(shared)
        tc = tokens[c * BC:(c + 1) * BC]
        m["idxw"] = prep_idx(tc, T)
        m["notm"] = prep_notm(tc, T)
        in_maps.append(m)
    res = run_bass_kernel_spmd(nc, in_maps, core_ids=list(range(NCORES)))
    out = np.concatenate([assemble_out(res.results[c]["outp"], BC) for c in range(NCORES)], axis=0)
    return out.astype(np.float32)

